# revision 1
# baseline (speedup 1.0000x reference)
"""Trainium2 Bass kernel for a 2-layer BiLSTM text classifier.

Computation (matches the reference):
  e = emb[x]  ->  BiLSTM1 (return sequences)  ->  BiLSTM2 (return last state)
  -> softmax(h @ Wd + bd)

Sharding: pure data-parallel over batch across 8 cores (16 rows/core),
weights replicated, no collectives.  Each core runs all 4 scans; the fwd
and bwd directions of a layer are interleaved as two independent
dependency chains so PE/ACT/DVE stay busy.

Key optimizations over the straightforward implementation:
  * Recurrent matmuls use an fp8-e3m4 STATIONARY operand (U weights) with
    a bf16 MOVING operand (h).  LDWEIGHTS bandwidth is the per-step PE
    bottleneck (32 U-tile reloads per step); fp8 + fast-weight-load cuts
    it ~4x vs f32.  h stays bf16 for accuracy and lives directly in the
    seqT sequence buffer for layer 1 (no separate h copy per step).
  * Gate order is permuted to (i,f,o,g) and the g-gate weights pre-scaled
    by 2 so ONE sigmoid serves all four gates (tanh(x) = 2*sigmoid(2x)-1).
  * Fused DVE cell update via scalar_tensor_tensor:
        fc  = f * c ;  igh = (sig_g - 0.5) * i ;  c = igh*2 + fc
  * Input projections x@W+b are precomputed per 32-step chunk as N=512
    matmuls (bf16 weights + bf16 eT for fast-weight-load); the PSUM->SBUF
    bias-copies alternate between ACT and DVE to balance engine load.
  * The embedding gather (indirect DMA + PE transpose) is interleaved
    with the phase-1 scan instead of running serially up front.
"""

import os

import numpy as np
import ml_dtypes

import concourse.bass as bass
import concourse.mybir as mybir
import concourse.tile as tile
from concourse import bacc
from concourse.bass_utils import run_bass_kernel_spmd
from concourse.masks import make_identity

# Problem dims (hardcoded per spec)
B, V, D, H, C = 128, 50000, 128, 256, 10
T = int(os.environ.get("KT", "512"))
NCORES = 8
BL = B // NCORES          # 16 batch rows per core
G = 4 * H                 # 1024 gate width
NM = G // 128             # 8 gate m-tiles
CHUNK = 32                # scan steps per xW chunk
NCH = T // CHUNK          # 16 chunks
NTOK = T * BL             # 8192 tokens per core, time-major (col = t*BL + j)
GCH = NTOK // 128         # 64 embedding gather chunks

F32 = mybir.dt.float32
BF16 = mybir.dt.bfloat16
I32 = mybir.dt.int32
F8 = mybir.dt.float8e3
BF = ml_dtypes.bfloat16
F8NP = ml_dtypes.float8_e3m4
AF = mybir.ActivationFunctionType
ALU = mybir.AluOpType

TRACE = False
LAST_RESULTS = None

# Keras gate order is i,f,g,o (each H wide).  Reorder columns to i,f,o,g so
# sigmoid gates are contiguous.  In the packed z layout blocks are:
# m=0,1 -> i ; m=2,3 -> f ; m=4,5 -> o ; m=6,7 -> g(tanh).
_PERM = np.concatenate(
    [np.arange(0, 2 * H), np.arange(3 * H, 4 * H), np.arange(2 * H, 3 * H)]
)


def _pack_k(w, kt, dt):
    """[kt*128, G] -> [128, kt, G] k-tile packing (partition-major)."""
    return np.ascontiguousarray(
        w.reshape(kt, 128, w.shape[1]).transpose(1, 0, 2)
    ).astype(dt)


def _prep_weights(inputs):
    """Host-side weight prep shared by all cores."""
    f32 = np.float32
    out = {}
    out["emb"] = np.ascontiguousarray(np.asarray(inputs["emb"], f32))
    # g-gate (cols 768:1024 post-perm) scaled by 2 so tanh(z_g) can be
    # computed as 2*sigmoid(2*z_g) - 1 with one fused sigmoid over all gates.
    for nm, kt, dt in [
        ("U1f", 2, F8NP), ("U1b", 2, F8NP), ("U2f", 2, F8NP), ("U2b", 2, F8NP),
        ("W2f", 4, BF), ("W2b", 4, BF),
    ]:
        w = np.asarray(inputs[nm], f32)[:, _PERM].copy()
        w[:, 3 * H:] *= 2.0
        out[nm.lower()] = _pack_k(w, kt, dt)
    for nm in ["W1f", "W1b"]:
        w = np.asarray(inputs[nm], f32)[:, _PERM].copy()
        w[:, 3 * H:] *= 2.0
        out[nm.lower()] = np.ascontiguousarray(w).astype(BF)
    for nm in ["b1f", "b1b", "b2f", "b2b"]:
        b = np.asarray(inputs[nm], f32)[_PERM].copy()
        b[3 * H:] *= 2.0
        out[nm.lower()] = np.ascontiguousarray(b.reshape(NM, 128).T).astype(f32)
    wd = np.asarray(inputs["Wd"], f32)  # [2H, C]
    out["wd"] = np.ascontiguousarray(
        wd.reshape(4, 128, C).transpose(1, 0, 2)
    ).astype(BF)
    out["bd"] = np.asarray(inputs["bd"], f32).reshape(1, C).astype(BF)
    return out


def _build():
    """Emit the Tile program (identical SPMD program for every core)."""
    nc = bacc.Bacc("TRN2", target_bir_lowering=False, debug=False,
                   num_devices=NCORES)

    # ---- DRAM I/O ----
    emb_d = nc.dram_tensor("emb", [V, D], F32, kind="ExternalInput")
    xidx_d = nc.dram_tensor("xidx", [128, GCH], I32, kind="ExternalInput")
    wdram = {}
    for nm in ["u1f", "u1b", "u2f", "u2b"]:
        wdram[nm] = nc.dram_tensor(nm, [128, 2, G], F8, kind="ExternalInput")
    for nm in ["w1f", "w1b"]:
        wdram[nm] = nc.dram_tensor(nm, [128, G], BF16, kind="ExternalInput")
    for nm in ["w2f", "w2b"]:
        wdram[nm] = nc.dram_tensor(nm, [128, 4, G], BF16, kind="ExternalInput")
    for nm in ["b1f", "b1b", "b2f", "b2b"]:
        wdram[nm] = nc.dram_tensor(nm, [128, NM], F32, kind="ExternalInput")
    wdram["wd"] = nc.dram_tensor("wd", [128, 4, C], BF16, kind="ExternalInput")
    wdram["bd"] = nc.dram_tensor("bd", [1, C], BF16, kind="ExternalInput")
    out_d = nc.dram_tensor("out", [BL, C], F32, kind="ExternalOutput")

    with tile.TileContext(nc) as tc, \
         tc.tile_pool(name="const", bufs=1) as const, \
         tc.tile_pool(name="work", bufs=2) as work, \
         tc.tile_pool(name="xwp", bufs=2) as xwp, \
         tc.tile_pool(name="psz", bufs=2, space="PSUM") as psz, \
         tc.tile_pool(name="psbig", bufs=2, space="PSUM") as psbig:

        # ---- load weights to SBUF ----
        sb = {}
        for nm, th in wdram.items():
            t_ = const.tile(list(th.shape), th.dtype, name=f"sb_{nm}",
                            tag=f"sb_{nm}")
            nc.sync.dma_start(out=t_[:], in_=th[:])
            sb[nm] = t_
        xidx = const.tile([128, GCH], I32, name="xidx_s", tag="xidx_s")
        nc.sync.dma_start(out=xidx[:], in_=xidx_d[:])

        ident = const.tile([128, 128], F32, name="ident", tag="ident")
        make_identity(nc, ident[:])
        ident_bf = const.tile([128, 128], BF16, name="ident_bf", tag="ident_bf")
        make_identity(nc, ident_bf[:])
        zero_h = const.tile([128, BL], BF16, name="zero_h", tag="zero_h")
        nc.vector.memset(zero_h[:], 0.0)
        ones_r = const.tile([1, BL], BF16, name="ones_r", tag="ones_r")
        nc.vector.memset(ones_r[:], 1.0)

        # big persistent buffers.  eT is split per xw-chunk so the
        # gather-copies (interleaved with the phase-1 scan) only create
        # dependencies against the xw matmuls of their own chunk.
        eTc = [const.tile([128, CHUNK * BL], BF16, name=f"eT{c}",
                          tag=f"eT{c}") for c in range(NCH)]
        seqT = const.tile([128, 4, NTOK], BF16, name="seqT", tag="seqT")
        c_st = {}
        for dn in ("f", "b"):
            c_st[dn] = const.tile([128, 2 * BL], F32, name=f"c_{dn}",
                                  tag=f"c_{dn}")

        # ---- embedding gather + transpose -> eT [D, NTOK] bf16 ----
        def gather_chunk(ch):
            erows = work.tile([128, D], F32, name="erows", tag="erows", bufs=3)
            nc.gpsimd.indirect_dma_start(
                out=erows[:],
                out_offset=None,
                in_=emb_d[:],
                in_offset=bass.IndirectOffsetOnAxis(
                    ap=xidx[:, ch:ch + 1], axis=0),
            )
            tp = psbig.tile([128, 128], F32, name="tp", tag="ps_misc")
            nc.tensor.transpose(out=tp[:], in_=erows[:], identity=ident[:])
            cc, j = divmod(ch, CHUNK * BL // 128)
            nc.vector.tensor_copy(out=eTc[cc][:, j * 128:(j + 1) * 128],
                                  in_=tp[:])

        # interleaved front/back order so both scan directions' xw chunks
        # have their tokens ready in time when gathers overlap phase 1
        gorder = []
        for i in range(GCH // 2):
            gorder += [i, GCH - 1 - i]
        PRO_G = min(24, GCH)      # prologue gathers (3 chunks each end)
        for gi in range(PRO_G):
            gather_chunk(gorder[gi])

        # ---- helpers ----
        def new_xw(dn):
            return xwp.tile([128, NM * CHUNK * BL], BF16, name=f"xw_{dn}",
                            tag=f"xw_{dn}")

        def xw_piece(layer, dn, cc, m, xw, alt):
            """One m-slice of the xW.T+b precompute for chunk cc."""
            cs = slice(cc * CHUNK * BL, (cc + 1) * CHUNK * BL)
            ps = psbig.tile([128, CHUNK * BL], F32, name="ps_xw", tag="ps_xw")
            if layer == 1:
                nc.tensor.matmul(
                    ps[:], lhsT=sb[f"w1{dn}"][:, m * 128:(m + 1) * 128],
                    rhs=eTc[cc][:], start=True, stop=True)
            else:
                for k in range(4):
                    nc.tensor.matmul(
                        ps[:],
                        lhsT=sb[f"w2{dn}"][:, k, m * 128:(m + 1) * 128],
                        rhs=seqT[:, k, cs],
                        start=(k == 0), stop=(k == 3))
            dst = xw[:, m * CHUNK * BL:(m + 1) * CHUNK * BL]
            bias = sb[f"b{layer}{dn}"][:, m:m + 1]
            if alt:
                nc.vector.tensor_scalar(out=dst, in0=ps[:], scalar1=bias,
                                        scalar2=None, op0=ALU.add)
            else:
                nc.scalar.activation(out=dst, in_=ps[:], func=AF.Identity,
                                     bias=bias, scale=1.0)

        def xw_chunk(layer, dn, cc):
            xw = new_xw(dn)
            for m in range(NM):
                xw_piece(layer, dn, cc, m, xw, alt=(m % 2 == 1))
            return xw

        def scan_pair(layer, steps):
            """One LSTM step for BOTH directions, stage-interleaved so the
            two dependency chains don't convoy on any engine's FIFO."""
            ctxs = []
            for dn, t, h_prev, xw, h_out, seq_out, hT_out in steps:
                u = sb[f"u{layer}{dn}"]
                z = psz.tile([128, NM * BL], F32, name=f"z_{dn}",
                             tag=f"z_{dn}", bufs=2)
                xw4 = xw.rearrange("p (m s b) -> p m s b", m=NM, s=CHUNK)
                tin = t % CHUNK
                # Seed PSUM with xw (identity matmul, start=True sets the
                # whole bank's has_written) so the 16 recurrent matmuls
                # accumulate on top.
                nc.tensor.matmul(z[:], lhsT=ident_bf[:],
                                 rhs=xw4[:, :, tin, :], start=True, stop=False)
                for m in range(NM):
                    for k in range(2):
                        nc.tensor.matmul(
                            z[:, m * BL:(m + 1) * BL],
                            lhsT=u[:, k, m * 128:(m + 1) * 128],
                            rhs=h_prev[k], start=False,
                            stop=(m == NM - 1 and k == 1))
                ctxs.append(dict(dn=dn, z=z, h_out=h_out, seq_out=seq_out,
                                 hT_out=hT_out))
            for x in ctxs:
                x["g"] = work.tile([128, NM * BL], BF16, name="g_" + x["dn"],
                                   tag=f"g_{x['dn']}", bufs=3)
                nc.scalar.activation(out=x["g"][:], in_=x["z"][:],
                                     func=AF.Sigmoid)
            for x in ctxs:
                # fc = f*c      (f gate = cols [2BL, 4BL))
                x["fc"] = work.tile([128, 2 * BL], F32, name="fc_" + x["dn"],
                                    tag=f"fc_{x['dn']}", bufs=3)
                nc.vector.tensor_mul(x["fc"][:], x["g"][:, 2 * BL:4 * BL],
                                     c_st[x["dn"]][:])
            for x in ctxs:
                # igh = (sig_g - 0.5) * i = i*g/2   (g raw sig = cols[6BL,8BL))
                x["igh"] = work.tile([128, 2 * BL], BF16, name="igh_" + x["dn"],
                                     tag=f"igh_{x['dn']}", bufs=3)
                nc.vector.scalar_tensor_tensor(
                    out=x["igh"][:], in0=x["g"][:, 6 * BL:8 * BL], scalar=0.5,
                    in1=x["g"][:, 0:2 * BL], op0=ALU.subtract, op1=ALU.mult)
            for x in ctxs:
                # c = igh*2 + fc
                nc.vector.scalar_tensor_tensor(
                    out=c_st[x["dn"]][:], in0=x["igh"][:], scalar=2.0,
                    in1=x["fc"][:], op0=ALU.mult, op1=ALU.add)
            for x in ctxs:
                x["th"] = work.tile([128, 2 * BL], BF16, name="th_" + x["dn"],
                                    tag=f"th_{x['dn']}", bufs=3)
                nc.scalar.activation(out=x["th"][:], in_=c_st[x["dn"]][:],
                                     func=AF.Tanh)
            for x in ctxs:
                o3 = x["g"][:, 4 * BL:6 * BL].rearrange("p (a b) -> p a b",
                                                        a=2)
                th3 = x["th"].rearrange("p (a b) -> p a b", a=2)
                if x["h_out"] is not None:
                    nc.vector.tensor_mul(x["h_out"], o3, th3)
                if x["seq_out"] is not None:
                    nc.vector.tensor_mul(x["seq_out"], o3, th3)
                if x["hT_out"] is not None:
                    nc.vector.tensor_mul(x["hT_out"], o3, th3)

        # ---- the two BiLSTM phases ----
        hT = {}
        for dn in ("f", "b"):
            hT[dn] = const.tile([128, 2, BL], BF16, name=f"hT_{dn}",
                                tag=f"hT_{dn}")

        def run_phase(layer):
            for dn in ("f", "b"):
                nc.vector.memset(c_st[dn][:], 0.0)
            xw_f = {0: xw_chunk(layer, "f", 0)}
            xw_b = {NCH - 1: xw_chunk(layer, "b", NCH - 1)}
            h = {"f": None, "b": None}
            pieces = []
            gnext = [PRO_G]
            for t in range(T):
                if t % CHUNK == 0:
                    # queue next chunks' pieces, spread 1/step below
                    pieces = []
                    cf = t // CHUNK + 1
                    cb = NCH - 2 - t // CHUNK
                    if cf < NCH:
                        xw_f[cf] = new_xw("f")
                        pf = [("f", cf, m, xw_f[cf]) for m in range(NM)]
                    else:
                        pf = []
                    if cb >= 0:
                        xw_b[cb] = new_xw("b")
                        pb = [("b", cb, m, xw_b[cb]) for m in range(NM)]
                    else:
                        pb = []
                    for a, b_ in zip(pf, pb):
                        pieces += [a, b_]
                    pieces += pf[len(pb):] + pb[len(pf):]
                if pieces:
                    dn_, cc_, m_, xwt = pieces.pop(0)
                    xw_piece(layer, dn_, cc_, m_, xwt,
                             alt=((t + m_) % 2 == 1))
                if layer == 1 and t % 4 == 2 and gnext[0] < GCH:
                    gather_chunk(gorder[gnext[0]])
                    gnext[0] += 1
                steps = []
                for dn, tt, xw in (("f", t, xw_f[t // CHUNK]),
                                   ("b", T - 1 - t,
                                    xw_b[(T - 1 - t) // CHUNK])):
                    if layer == 1:
                        # h history lives in seqT directly (both bf16):
                        # one DVE write per step.
                        ks = 0 if dn == "f" else 2
                        if t == 0:
                            hp = [zero_h[:], zero_h[:]]
                        elif dn == "f":
                            hp = [seqT[:, k, (tt - 1) * BL:tt * BL]
                                  for k in range(2)]
                        else:
                            hp = [seqT[:, 2 + k, (tt + 1) * BL:(tt + 2) * BL]
                                  for k in range(2)]
                        so = seqT[:, ks:ks + 2, tt * BL:(tt + 1) * BL]
                        steps.append((dn, tt, hp, xw, so, None, None))
                        continue
                    if h[dn] is None:
                        hp = [zero_h[:], zero_h[:]]
                    else:
                        hp = [h[dn][:, k, :] for k in range(2)]
                    hn = work.tile([128, 2, BL], BF16, name=f"h{layer}_{dn}",
                                   tag=f"h{layer}_{dn}", bufs=3)
                    hTo = hT[dn][:, :, :] if t == T - 1 else None
                    steps.append((dn, tt, hp, xw, hn[:, :, :], None, hTo))
                    h[dn] = hn
                scan_pair(layer, steps)

        run_phase(1)
        run_phase(2)

        # ---- dense + softmax ----
        ps = psbig.tile([BL, C], F32, name="ps_d", tag="ps_misc")
        for ki, (dn, k) in enumerate([("f", 0), ("f", 1), ("b", 0), ("b", 1)]):
            nc.tensor.matmul(ps[:], lhsT=hT[dn][:, k, :], rhs=sb["wd"][:, ki, :],
                             start=(ki == 0), stop=False)
        nc.tensor.matmul(ps[:], lhsT=ones_r[:], rhs=sb["bd"][:],
                         start=False, stop=True)
        mx = work.tile([BL, 1], F32, name="mx", tag="mx")
        nc.vector.reduce_max(out=mx[:], in_=ps[:], axis=mybir.AxisListType.X)
        mxn = work.tile([BL, 1], F32, name="mxn", tag="mxn")
        nc.vector.tensor_scalar_mul(mxn[:], mx[:], -1.0)
        ex = work.tile([BL, C], F32, name="ex", tag="ex")
        sm = work.tile([BL, 1], F32, name="sm", tag="sm")
        nc.scalar.activation(out=ex[:], in_=ps[:], func=AF.Exp,
                             bias=mxn[:, 0:1], scale=1.0, accum_out=sm[:])
        rs = work.tile([BL, 1], F32, name="rs", tag="rs")
        nc.vector.reciprocal(rs[:], sm[:])
        osm = work.tile([BL, C], F32, name="osm", tag="osm")
        nc.vector.tensor_scalar_mul(osm[:], ex[:], rs[:, 0:1])
        nc.sync.dma_start(out=out_d[:], in_=osm[:])

    nc.compile()
    return nc


_CACHE = {}


def make_in_maps(inputs):
    w = _prep_weights(inputs)
    x = np.asarray(inputs["x"], np.int32)[:, :T]  # [B, T]
    in_maps = []
    for core in range(NCORES):
        xc = x[core * BL:(core + 1) * BL]            # [BL, T]
        tm = np.ascontiguousarray(xc.T).reshape(-1)  # time-major [T*BL]
        xi = np.ascontiguousarray(tm.reshape(GCH, 128).T).astype(np.int32)
        m = {"xidx": xi}
        m["emb"] = w["emb"]
        for nm in ["u1f", "u1b", "u2f", "u2b", "w1f", "w1b", "w2f", "w2b",
                   "b1f", "b1b", "b2f", "b2b", "wd", "bd"]:
            m[nm] = w[nm]
        in_maps.append(m)
    return in_maps


def get_nc():
    if "nc" not in _CACHE:
        _CACHE["nc"] = _build()
    return _CACHE["nc"]


def kernel(**inputs):
    global LAST_RESULTS
    nc = get_nc()
    in_maps = make_in_maps(inputs)
    res = run_bass_kernel_spmd(nc, in_maps, core_ids=list(range(NCORES)),
                               trace=TRACE)
    LAST_RESULTS = res
    return np.concatenate([r["out"] for r in res.results], axis=0)

